# revision 2
# baseline (speedup 1.0000x reference)
"""CAML multi-label attention kernel v2 for Trainium2 (8 NeuronCores).

Reference (B=8, W=1000, V=50000, E=100, C=50, K=3, L=18000):
    emb    = W_embed[x]                            (B, W, E)
    H      = tanh(conv1d(emb, conv_w) + conv_b)    (B, W, C)
    scores = einsum("lc,bwc->blw", u_w, H)
    attns  = softmax(scores, axis=w)
    m      = einsum("blw,bwc->blc", attns, H)
    out    = sigmoid(sum(out_w * m, axis=c) + out_b)   (B, L)

v2 design:
  - BATCH sharding: core b computes batch b for ALL 18000 labels (prologue
    runs once per core instead of 8x).
  - u_w host-prescaled by 11.5416 = 8/ln2 so PSUM scores are in "e4m3
    exponent bit" units. PSUM can only be drained by ScalarE/DVE, so exp is
    split between them, alternating per chunk-pair so both run concurrently:
      * ScalarE: activation(Exp, scale) -> fp8e4
      * DVE: tensor_scalar(mult 0.5, add 56.04) -> int8 = e4m3 BITS of
        exp(s) (Schraudolph with truncation centering); bitcast to fp8.
  - mm1/mm2 in fp8 DoubleRow. The po contraction halves are stride-0
    broadcast views (operands stored once; result is 2x, folded into the
    exp constants), so uwt DMA/SBUF is halved.
  - mm2 packs 2 w-chunks per pass (contraction 256): 4 passes instead of 8.
  - pooled m -> label-partition layout via PE transposes; PSUM->SBUF staging
    copies alternate ScalarE/DVE per block.
  - epilogue dot+reduce runs on the otherwise-idle Pool engine in 3 chunks
    interleaved with the main loop; only the tiny tail is on DVE/ScalarE.
"""

import os

import numpy as np

try:
    import concourse.bass as bass
except ImportError:
    import sys

    sys.path.insert(0, "/opt/trn_rl_repo")
    import concourse.bass as bass

import concourse.bacc as bacc
import concourse.tile as tile
from concourse import mybir
from concourse.bass import IndirectOffsetOnAxis
from concourse.bass_utils import run_bass_kernel_spmd
from concourse.masks import make_identity

FP = mybir.dt.float32
BF = mybir.dt.bfloat16
F8 = mybir.dt.float8e4
I8 = mybir.dt.int8
AF = mybir.ActivationFunctionType
DR = mybir.MatmulPerfMode.DoubleRow
ALU = mybir.AluOpType

B, W, V, E, C, K, L = 8, 1000, 50000, 100, 50, 3, 18000
NCORES = 8
WPAD = 1024
NCI = WPAD // 128  # 8 w-chunks
LT = 141  # 128-label tiles (18048 padded)
LTPAD = LT * 128
NLB = 36  # label blocks: 35 x 512 + 1 x 80
SCALE = 11.5416  # 8 / ln2
DELTA = float(os.environ.get("DELTA", "56.0416"))  # trunc-convert centering
EXP_PAT = os.environ.get("EXP_PAT", "SDSD")  # exp engine per pair
# fp8 mm1 fails walrus codegen ("s3_lw_dual_fp8_restrictions": dual-fp8
# ldweights needs 128 contraction partitions + real strides); bf16 mm1 is
# off the critical path anyway (exp engines bound) so it costs nothing.
MM1 = os.environ.get("MM1", "bf16")  # fp8 | bf16
COPYQ = os.environ.get("COPYQ", "alt")  # alt | scalar | vector
EPIL = os.environ.get("EPIL", "pool")  # pool | vector


def _lw(lb):
    return 512 if lb < 35 else 80


def build_nc(num_devices: int, repeat: int = 1):
    nc = bacc.Bacc(
        "TRN2", target_bir_lowering=False, debug=False, num_devices=num_devices
    )
    x_idx = nc.dram_tensor("x_idx", [128, NCI], mybir.dt.int32, kind="ExternalInput").ap()
    wemb = nc.dram_tensor("wemb", [V, E], FP, kind="ExternalInput").ap()
    convwt = nc.dram_tensor("convwt", [E, K * C], BF, kind="ExternalInput").ap()
    convb = nc.dram_tensor("convb", [C, 1], FP, kind="ExternalInput").ap()
    if MM1 == "fp8":
        uwt = nc.dram_tensor("uwt", [50, L], F8, kind="ExternalInput").ap()
    else:
        uwt = nc.dram_tensor("uwt", [50, L], BF, kind="ExternalInput").ap()
    owp = nc.dram_tensor("owp", [128, LT * C], FP, kind="ExternalInput").ap()
    obp = nc.dram_tensor("obp", [128, LT], FP, kind="ExternalInput").ap()
    out = nc.dram_tensor("out", [128, LT], FP, kind="ExternalOutput").ap()

    with tile.TileContext(nc) as tc:
        for _ in range(repeat):
            _body(tc, nc, x_idx, wemb, convwt, convb, uwt, owp, obp, out)
    nc.compile()
    return nc


def _body(tc, nc, x_idx, wemb, convwt, convb, uwt, owp, obp, out):
    fp8 = MM1 == "fp8"
    # with the stride-0 po broadcast both mm1 operands are used twice, so
    # PSUM sc = 2x the "bit-unit" scores
    sc_mul = 0.5 if fp8 else 1.0
    with (
        tc.tile_pool(name="const", bufs=1) as constp,
        tc.tile_pool(name="work", bufs=2) as workp,
        tc.tile_pool(name="expp", bufs=3) as expp,
    ):
        ident = constp.tile([128, 128], FP, name="ident")
        make_identity(nc, ident)
        ident_bf = constp.tile([128, 128], BF, name="ident_bf")
        make_identity(nc, ident_bf)

        uwt_s = constp.tile([50, L], F8 if fp8 else BF, name="uwt_s")
        # chunked load so lb 0 can start before the whole tensor lands
        for q0 in range(0, L, 4500):
            nc.sync.dma_start(
                out=uwt_s[:, q0 : q0 + 4500], in_=uwt[:, q0 : q0 + 4500]
            )
        convwt_s = constp.tile([E, K * C], BF, name="convwt_s")
        nc.sync.dma_start(out=convwt_s, in_=convwt)
        convb_s = constp.tile([C, 1], FP, name="convb_s")
        nc.sync.dma_start(out=convb_s, in_=convb)
        owp_s = constp.tile([128, LT * C], FP, name="owp_s")
        nc.sync.dma_start(out=owp_s, in_=owp)
        obp_s = constp.tile([128, LT], FP, name="obp_s")
        nc.sync.dma_start(out=obp_s, in_=obp)
        idx_s = constp.tile([128, NCI], mybir.dt.int32, name="idx_s")
        nc.sync.dma_start(out=idx_s, in_=x_idx)

        Hc8 = constp.tile([50, WPAD], F8, name="Hc8")
        Hbf = constp.tile([50, WPAD], BF, name="Hbf")
        # dual-fp8 ldweights needs aligned stationary columns: pad 51 -> 64
        CP = 64
        Haug8 = constp.tile([128, 4 * 2 * CP], F8, name="Haug8")
        mlt = constp.tile([128, LT * 64], FP, name="mlt")
        dsum = constp.tile([128, LT], FP, name="dsum")
        prodb = constp.tile([128, LT * C], FP, name="prodb")

        nc.gpsimd.memset(Hc8.bitcast(I8)[:, W:WPAD], 0)
        nc.gpsimd.memset(Hbf[:, W:WPAD], 0.0)
        # pad-label region of mlt (tile 140): denom 1, rest 0; the lb=35
        # copy later overwrites the valid partitions 0:80
        mlt3 = mlt.rearrange("p (t s) -> p t s", s=64)
        nc.gpsimd.memset(mlt3[:, 140, :], 0.0)
        nc.gpsimd.memset(mlt3[:, 140, C : C + 1], 1.0)

        # ---------------- prologue: gather, conv, layouts ----------------
        # PSUM pools are scoped manually: the prologue pool's bank is
        # released before the main loop's pools take 6+1+1 banks.
        psP_cm = tc.tile_pool(name="psP", bufs=1, space="PSUM")
        psP = psP_cm.__enter__()
        embP = workp.tile([E, 1032], BF, tag="embP", name="embP", bufs=1)
        nc.gpsimd.memset(embP[:, 0:1], 0.0)
        nc.gpsimd.memset(embP[:, 1001:1032], 0.0)
        for ci in range(NCI):
            emb_g = workp.tile([128, E], BF, tag="embg", name="emb_g", bufs=12)
            nc.gpsimd.indirect_dma_start(
                out=emb_g[:, :],
                out_offset=None,
                in_=wemb[:, :],
                in_offset=IndirectOffsetOnAxis(ap=idx_s[:, ci : ci + 1], axis=0),
            )
            pt = psP.tile([128, 128], BF, tag="pp", name="pt")
            nc.tensor.transpose(out=pt[:E, :], in_=emb_g[:, :], identity=ident_bf[:, :])
            cw = min(128, W - ci * 128)
            nc.vector.tensor_copy(
                out=embP[:, 1 + ci * 128 : 1 + ci * 128 + cw], in_=pt[:E, :cw]
            )

        for w0, cw in ((0, 512), (512, W - 512)):
            pm = psP.tile([C, 512], FP, tag="pp", name="convps")
            for k in range(K):
                nc.tensor.matmul(
                    out=pm[:C, :cw],
                    lhsT=convwt_s[:, k * C : (k + 1) * C],
                    rhs=embP[:, w0 + k : w0 + k + cw],
                    start=(k == 0),
                    stop=(k == K - 1),
                )
            nc.scalar.activation(
                out=Hc8[:C, w0 : w0 + cw],
                in_=pm[:C, :cw],
                func=AF.Tanh,
                bias=convb_s[:, 0:1],
            )
            nc.scalar.activation(
                out=Hbf[:C, w0 : w0 + cw],
                in_=pm[:C, :cw],
                func=AF.Tanh,
                bias=convb_s[:, 0:1],
            )

        # Haug8[p, pair, po, 0:50] = H chunk transposed; col 50 = 1.0 (valid w)
        Haug8v = Haug8.rearrange("p (pr po c) -> p pr po c", pr=4, po=2)
        o8 = Haug8.bitcast(I8).rearrange("p (pr po c) -> p pr po c", pr=4, po=2)
        nc.gpsimd.memset(Haug8.bitcast(I8), 0)  # pad cols 51:64 contribute 0
        for ci in range(NCI):
            pt2 = psP.tile([128, 64], BF, tag="pp", name="pt2")
            nc.tensor.transpose(
                out=pt2[:, :C],
                in_=Hbf[:C, ci * 128 : (ci + 1) * 128],
                identity=ident_bf[:C, :C],
            )
            nc.scalar.activation(
                out=Haug8v[:, ci // 2, ci % 2, 0:C], in_=pt2[:, :C], func=AF.Copy
            )
            if ci < NCI - 1:
                nc.gpsimd.memset(o8[:, ci // 2, ci % 2, C : C + 1], 56)
            else:
                nc.gpsimd.memset(o8[:, ci // 2, ci % 2, C : C + 1], 0)
                nc.gpsimd.memset(o8[: W - 896, ci // 2, ci % 2, C : C + 1], 56)

        if fp8:
            Hd8v = Hc8.rearrange("p (x w) -> p x w", x=1).broadcast_to((50, 2, WPAD))
            uwv = uwt_s.rearrange("p (x l) -> p x l", x=1).broadcast_to((50, 2, L))

        psP_cm.__exit__(None, None, None)
        psA_cm = tc.tile_pool(name="psA", bufs=3, space="PSUM")
        psA = psA_cm.__enter__()  # sc: 2 banks x3
        psB_cm = tc.tile_pool(name="psB", bufs=1, space="PSUM")
        psB = psB_cm.__enter__()  # ma: 1 bank
        psT_cm = tc.tile_pool(name="psT", bufs=1, space="PSUM")
        psT = psT_cm.__enter__()  # ptm: 1 bank

        # -------- epilogue chunks (mul+reduce on Pool, interleaved) --------
        ow3 = owp_s.rearrange("p (t c) -> p t c", c=C)
        prod3 = prodb.rearrange("p (t c) -> p t c", c=C)
        EPI_ENG = nc.gpsimd if EPIL == "pool" else nc.vector

        def epi_chunk(t0, t1):
            EPI_ENG.tensor_mul(
                out=prod3[:, t0:t1], in0=mlt3[:, t0:t1, 0:C], in1=ow3[:, t0:t1]
            )
            nc.vector.tensor_reduce(
                out=dsum[:, t0:t1],
                in_=prod3[:, t0:t1],
                axis=mybir.AxisListType.X,
                op=ALU.add,
            )

        # ---------------------- main label loop ---------------------------
        def _copy(eng, dst, src):
            if eng == "scalar":
                nc.scalar.activation(out=dst, in_=src, func=AF.Copy)
            else:
                nc.vector.tensor_copy(out=dst, in_=src)

        def lb_post(ma_u, lb_u, LW_u):
            if COPYQ == "alt":
                e1_, e2_ = ("scalar", "vector") if lb_u % 2 else ("vector", "scalar")
            else:
                e1_ = e2_ = COPYQ
            msb = workp.tile([C + 1, 512], FP, tag="msb", name="msb")
            _copy(e1_, msb[:, :LW_u], ma_u[: C + 1, :LW_u])
            nq = (LW_u + 127) // 128
            ptm = psT.tile([128, 256], FP, tag="pt", name="ptm")
            for q in range(nq):
                cq = min(128, LW_u - q * 128)
                nc.tensor.transpose(
                    out=ptm[:cq, q * 64 : q * 64 + C + 1],
                    in_=msb[:, q * 128 : q * 128 + cq],
                    identity=ident[: C + 1, : C + 1],
                )
            dst = mlt3[:, lb_u * 4 : lb_u * 4 + nq, 0 : C + 1]
            src = ptm.rearrange("p (q s) -> p q s", s=64)[:, 0:nq, 0 : C + 1]
            if LW_u < 512:
                dst = mlt3[: LW_u - (nq - 1) * 128, lb_u * 4 : lb_u * 4 + nq, 0 : C + 1]
                src = src[: LW_u - (nq - 1) * 128]
            _copy(e2_, dst, src)

        def emit_mm2(u):
            ma_u, ex_u, lb_u, LW_u, pair_u = u
            exv = ex_u[:, : 2 * LW_u].rearrange("p (po n) -> p po n", po=2)
            nc.tensor.matmul(
                out=ma_u[:, :LW_u],
                lhsT=Haug8v[:, pair_u, :, :],
                rhs=exv,
                start=(pair_u == 0),
                stop=(pair_u == 3),
                perf_mode=DR,
            )
            if pair_u == 3:
                lb_post(ma_u, lb_u, LW_u)

        pend = None
        for lb in range(NLB):
            lb0 = lb * 512
            LW = _lw(lb)
            ma = psB.tile([CP, 512], FP, tag="maug", name="ma")
            for pair in range(4):
                sc = psA.tile([128, 1024], FP, tag="sc", name="sc")
                ex = expp.tile([128, 1024], F8, tag="ex", name="ex")
                for po in range(2):
                    ci = pair * 2 + po
                    if fp8:
                        nc.tensor.matmul(
                            out=sc[:, po * LW : (po + 1) * LW],
                            lhsT=Hd8v[:, :, ci * 128 : (ci + 1) * 128],
                            rhs=uwv[:, :, lb0 : lb0 + LW],
                            start=True,
                            stop=True,
                            perf_mode=DR,
                        )
                    else:
                        nc.tensor.matmul(
                            out=sc[:, po * LW : (po + 1) * LW],
                            lhsT=Hbf[:C, ci * 128 : (ci + 1) * 128],
                            rhs=uwt_s[:C, lb0 : lb0 + LW],
                            start=True,
                            stop=True,
                        )
                if EXP_PAT[(lb * 4 + pair) % len(EXP_PAT)] == "S":
                    nc.scalar.activation(
                        out=ex[:, : 2 * LW],
                        in_=sc[:, : 2 * LW],
                        func=AF.Exp,
                        scale=sc_mul / SCALE,
                    )
                else:
                    nc.vector.tensor_scalar(
                        ex.bitcast(I8)[:, : 2 * LW],
                        sc[:, : 2 * LW],
                        sc_mul,
                        DELTA,
                        ALU.mult,
                        ALU.add,
                    )
                if pend is not None:
                    emit_mm2(pend)
                pend = (ma, ex, lb, LW, pair)
            # lb_post for block b is emitted one pair late (during block b+1),
            # so tiles 4b..4b+3 are only written once block b+1's pairs ran
            if lb in (12, 24):
                t0 = 0 if lb == 12 else 47
                epi_chunk(t0, t0 + 47)
        if pend is not None:
            emit_mm2(pend)
            pend = None
        epi_chunk(94, LT)

        # ------------------------- epilogue tail ---------------------------
        rs = workp.tile([128, LT], FP, tag="rs", name="rs", bufs=1)
        nc.vector.reciprocal(out=rs, in_=mlt3[:, :, C : C + 1])
        dz = workp.tile([128, LT], FP, tag="dz", name="dz", bufs=1)
        nc.vector.tensor_mul(out=dz, in0=dsum, in1=rs)
        zt = workp.tile([128, LT], FP, tag="zt", name="zt", bufs=1)
        nc.vector.tensor_add(out=zt, in0=dz, in1=obp_s)
        ez = workp.tile([128, LT], FP, tag="ez", name="ez", bufs=1)
        nc.scalar.activation(out=ez, in_=zt, func=AF.Exp, scale=-1.0)
        e1 = workp.tile([128, LT], FP, tag="e1", name="e1", bufs=1)
        nc.vector.tensor_scalar_add(e1, ez, 1.0)
        osb = workp.tile([128, LT], FP, tag="osb", name="osb", bufs=1)
        nc.vector.reciprocal(out=osb, in_=e1)
        nc.sync.dma_start(out=out, in_=osb)
        psT_cm.__exit__(None, None, None)
        psB_cm.__exit__(None, None, None)
        psA_cm.__exit__(None, None, None)


def host_prep(inputs):
    """Full inputs -> list of 8 per-core input maps (core c = batch c)."""
    import ml_dtypes

    E4 = ml_dtypes.float8_e4m3
    x = np.asarray(inputs["x"]).astype(np.int32)
    wemb = np.ascontiguousarray(np.asarray(inputs["W_embed"], dtype=np.float32))
    conv_w = np.asarray(inputs["conv_w"], dtype=np.float32)
    conv_b = np.asarray(inputs["conv_b"], dtype=np.float32)
    u_w = np.asarray(inputs["u_w"], dtype=np.float32)
    out_w = np.asarray(inputs["out_w"], dtype=np.float32)
    out_b = np.asarray(inputs["out_b"], dtype=np.float32)

    xp = np.zeros((B, WPAD), np.int32)
    xp[:, :W] = x

    convwt = np.ascontiguousarray(
        np.concatenate([conv_w[:, :, k].T for k in range(K)], axis=1)
    ).astype(ml_dtypes.bfloat16)
    convbp = np.ascontiguousarray(conv_b.reshape(C, 1))

    if MM1 == "fp8":
        uwt = np.ascontiguousarray(
            np.clip(u_w * SCALE, -240, 240).astype(E4).T
        )  # (50, L)
    else:
        uwt = np.ascontiguousarray((u_w * SCALE).T.astype(ml_dtypes.bfloat16))

    ow_pad = np.zeros((LTPAD, C), np.float32)
    ow_pad[:L] = out_w
    ob_pad = np.zeros(LTPAD, np.float32)
    ob_pad[:L] = out_b
    owp = np.ascontiguousarray(
        ow_pad.reshape(LT, 128, C).transpose(1, 0, 2).reshape(128, LT * C)
    )
    obp = np.ascontiguousarray(ob_pad.reshape(LT, 128).T)

    in_maps = []
    for c in range(NCORES):
        idx = np.ascontiguousarray(xp[c].reshape(NCI, 128).T)
        in_maps.append(
            {
                "x_idx": idx,
                "wemb": wemb,
                "convwt": convwt,
                "convb": convbp,
                "uwt": uwt,
                "owp": owp,
                "obp": obp,
            }
        )
    return in_maps


def unshard(outs):
    """outs: list of 8 arrays (128, LT) -> (B, L)."""
    rows = [np.asarray(o).T.reshape(LTPAD)[:L] for o in outs]
    return np.ascontiguousarray(np.stack(rows, axis=0), dtype=np.float32)


_NC = None
LAST_RESULTS = None


def kernel(**inputs) -> np.ndarray:
    global _NC, LAST_RESULTS
    in_maps = host_prep(inputs)
    if _NC is None:
        _NC = build_nc(num_devices=NCORES)
    trace = bool(int(os.environ.get("KERNEL_TRACE", "0")))
    res = run_bass_kernel_spmd(_NC, in_maps, core_ids=list(range(NCORES)), trace=trace)
    LAST_RESULTS = res
    outs = [res.results[i]["out"] for i in range(NCORES)]
    return unshard(outs)
